# revision 21
# baseline (speedup 1.0000x reference)
"""Trainium2 8-core kernel for causal multi-head attention block.

Module: qkv = x @ w_qkv + b_qkv ; causal MHA (16 heads, hd=64) ; out = attn @ w_out + b_out
Shapes: x (4, 2048, 1024), out (4, 2048, 1024), f32.

Sharding (8 cores): tensor-parallel over heads — core c owns heads {2c, 2c+1}
for ALL batches (QKV columns sharded head-wise). After attention, an 8-way
AllToAll converts head-sharding to token-sharding: token group of core c is
(batch c//2, token half c%2). Each core then runs the out projection for its
1024 tokens and outputs its (1024, 1024) f32 slice; the host reassembles.

Device algorithm per core (matmuls bf16, f32 PSUM accumulation):
  1. Q^T, K^T (head-cols on partitions) and V (tokens on partitions, with a
     constant ones-column appended at col 65) from streamed x^T tiles
  2. scores S^T[k,q] = K^T.T @ Q^T per head (2 heads packed in the PE array
     via row tiling, K=64 each); exp on ScalarE with the 1/8 softmax scale
     folded in; causal mask via gpsimd memset/affine_select on diagonal tiles
  3. PV: out^T[hd,q] + l[q] (row 64, from the ones-column of V) accumulated
     over k-blocks in PSUM; normalize by 1/l; bf16 to the AllToAll buffer
  4. two AllToAlls (one per within-half column chunk) so comm overlaps the
     second half of attention and the out projection
  5. out projection for my token group + bias (bias via K=1 matmul), f32 out
"""

import os
import numpy as np
import ml_dtypes

B, N, C, H, HD = 4, 2048, 1024, 16, 64
SCALE = HD ** -0.5
P = 128
CB = C // P               # 8 contraction blocks
NKB = N // P              # 16 key blocks per batch
NQC = N // 512            # 4 query chunks per batch
TOK_G = 1024              # tokens per core after A2A (output slice rows)
NHC = 2                   # heads per core
VC = NHC * HD             # 128 v columns per core

BF16 = ml_dtypes.bfloat16

_CACHE = {}
_NO_FEED = bool(int(os.environ.get("KERNEL_NO_FEED", "0")))
_OLD_DRAIN = bool(int(os.environ.get("KERNEL_OLD_DRAIN", "0")))


def _build_nc():
    import concourse.bass as bass
    import concourse.tile as tile
    from concourse import bacc, mybir
    from concourse.bass import ts, ds
    from contextlib import ExitStack

    FP = mybir.dt.float32
    BF = mybir.dt.bfloat16
    EXP = mybir.ActivationFunctionType.Exp
    IDENT = mybir.ActivationFunctionType.Identity

    nc = bacc.Bacc(num_devices=8)

    # per-core inputs
    xT_p = nc.declare_dram_parameter("xT", [B, C, N], BF, isOutput=False)
    wqk_p = nc.declare_dram_parameter("wqk", [C, 2 * VC], BF, isOutput=False)
    wv_p = nc.declare_dram_parameter("wv", [C, VC], BF, isOutput=False)
    wout_p = nc.declare_dram_parameter("wout", [C, C], BF, isOutput=False)
    bqk_p = nc.declare_dram_parameter("bqk", [P, 2], FP, isOutput=False)
    bv_p = nc.declare_dram_parameter("bv", [1, VC], BF, isOutput=False)
    bout_p = nc.declare_dram_parameter("bout", [1, C], BF, isOutput=False)
    out_p = nc.declare_dram_parameter("out", [TOK_G, C], FP, isOutput=True)

    # A2A buffers: shard s carries my 128 feature rows for core s's tokens.
    # Split into two column phases for comm/compute overlap.
    a2a_in = [
        nc.dram_tensor(f"a2a_in{ph}", [8, P, 512], BF) for ph in range(2)
    ]
    a2a_out = [
        nc.dram_tensor(f"a2a_out{ph}", [8, P, 512], BF) for ph in range(2)
    ]

    with tile.TileContext(nc) as tc, ExitStack() as ctx:
        const = ctx.enter_context(tc.tile_pool(name="const", bufs=1))
        big = ctx.enter_context(tc.tile_pool(name="big", bufs=1))
        xt_pool = ctx.enter_context(tc.tile_pool(name="xtp", bufs=3))
        pt_pool = ctx.enter_context(tc.tile_pool(name="ptp", bufs=4))
        # PSUM budget (8 banks): psk 2x[128,512]=2, ps 2x[128,2,512]=4, po 2x[65,512]=2
        psk_pool = ctx.enter_context(tc.tile_pool(name="pskp", bufs=2, space="PSUM"))
        ps_pool = ctx.enter_context(tc.tile_pool(name="psp", bufs=2, space="PSUM"))
        po_pool = ctx.enter_context(tc.tile_pool(name="pop", bufs=2, space="PSUM"))
        misc = ctx.enter_context(tc.tile_pool(name="misc", bufs=6))
        outp = ctx.enter_context(tc.tile_pool(name="outp", bufs=4))

        wqk = const.tile([P, CB, 2 * VC], BF)
        nc.sync.dma_start(wqk, wqk_p.rearrange("(cb p) c -> p cb c", p=P))
        wv = const.tile([P, CB, VC], BF)
        nc.sync.dma_start(wv, wv_p.rearrange("(cb p) c -> p cb c", p=P))
        bqk = const.tile([P, 2], FP)
        nc.sync.dma_start(bqk, bqk_p[:])
        bv = const.tile([1, VC], BF)
        nc.sync.dma_start(bv, bv_p[:])
        bvb = const.tile([P, VC], BF)
        nc.gpsimd.partition_broadcast(bvb, bv)

        # per-(batch, 512-token-chunk) Q^T/K^T and V tiles for fine deps
        qk_t = [
            [big.tile([P, 2, 512], BF, name=f"qkT{b}_{t}") for t in range(NQC)]
            for b in range(B)
        ]
        v_t = []
        for b in range(B):
            row = []
            for t in range(NQC):
                vt = big.tile([P, 4, NHC, HD + 1], BF, name=f"v{b}_{t}")
                nc.vector.memset(vt[:, :, :, HD : HD + 1], 1.0)
                row.append(vt)
            v_t.append(row)

        # ---- QKV work units (one unit = one 8-MM chain + epilogue) -------
        def qkv_unit(b, tch, kind, idx):
            """kind 'qk': idx in (0,1); kind 'v': idx in 0..3"""
            xt = xt_cache.get((b, tch))
            if xt is None:
                xt = xt_pool.tile([P, CB, 512], BF, tag="xt", name=f"xt{b}_{tch}")
                nc.sync.dma_start(
                    xt, xT_p[b, :, ts(tch, 512)].rearrange("(cb p) t -> p cb t", p=P)
                )
                xt_cache[(b, tch)] = xt
            if kind == "qk":
                qk = idx
                psq = psk_pool.tile([P, 512], FP, tag="psk", name=f"psq{b}_{tch}_{qk}")
                for kb in range(CB):
                    nc.tensor.matmul(
                        psq,
                        lhsT=wqk[:, kb, ts(qk, P)],
                        rhs=xt[:, kb, :],
                        start=(kb == 0),
                        stop=(kb == CB - 1),
                        skip_group_check=True,
                    )
                nc.vector.tensor_scalar_add(
                    qk_t[b][tch][:, qk, :], psq, bqk[:, qk : qk + 1]
                )
            else:
                tb4 = idx
                psv = psk_pool.tile([P, 512], FP, tag="psk", name=f"psv{b}_{tch}_{tb4}")
                for kb in range(CB):
                    nc.tensor.matmul(
                        psv[:, :VC],
                        lhsT=xt[:, kb, ts(tb4, P)],
                        rhs=wv[:, kb, :],
                        start=(kb == 0),
                        stop=(kb == CB - 1),
                        skip_group_check=True,
                    )
                nc.vector.tensor_tensor(
                    v_t[b][tch][:, tb4, :, 0:HD],
                    psv[:, :VC].rearrange("p (h d) -> p h d", h=NHC),
                    bvb.rearrange("p (h d) -> p h d", h=NHC),
                    mybir.AluOpType.add,
                )

        xt_cache = {}
        # unit list in order; done_upto[(b, tch)] = index after last unit of (b,tch)
        units = []
        unit_done = {}
        for b in range(B):
            for tch in range(NQC):
                for qk in range(2):
                    units.append((b, tch, "qk", qk))
                for tb4 in range(4):
                    units.append((b, tch, "v", tb4))
                unit_done[(b, tch)] = len(units)
        next_unit = [0]

        def flush_units(upto):
            while next_unit[0] < upto:
                qkv_unit(*units[next_unit[0]])
                next_unit[0] += 1

        def pop_units(n):
            flush_units(min(len(units), next_unit[0] + n))

        # ---- attention ----------------------------------------------------
        def emit_attention(b, j, ph, feed):
            # needs Q chunk j, K/V chunks 0..j of batch b
            flush_units(unit_done[(b, j)])
            po2 = [
                po_pool.tile([HD + 1, 512], FP, tag="po", name=f"po{b}_{j}_{k}")
                for k in range(2)
            ]
            nkb = 4 * j + 4
            for i in range(nkb):
                m = max(0, i - 4 * j)
                q0loc = P * m  # q offset within chunk j
                w = 512 - P * m
                pss = ps_pool.tile([P, 2, 512], FP, tag="ps", name=f"pss{b}_{j}_{i}")
                for hh in range(2):
                    rlo = 64 * hh
                    nc.tensor.matmul(
                        pss[:, hh, P * m : 512],
                        lhsT=qk_t[b][i // 4][:, 1, :][rlo : rlo + 64, ts(i % 4, P)],
                        rhs=qk_t[b][j][:, 0, :][rlo : rlo + 64, ds(q0loc, w)],
                        start=True,
                        stop=True,
                        skip_group_check=True,
                    )
                pt = pt_pool.tile([P, 2, 512], BF, tag="pt", name=f"pt{b}_{j}_{i}")
                nc.scalar.activation(
                    pt[:, :, P * m : 512], pss[:, :, P * m : 512], EXP, scale=SCALE
                )
                if i >= 4 * j:
                    nc.gpsimd.affine_select(
                        out=pt[:, :, P * m : P * m + P],
                        in_=pt[:, :, P * m : P * m + P],
                        compare_op=mybir.AluOpType.is_ge,
                        fill=0.0,
                        base=0,
                        pattern=[[0, 2], [1, P]],
                        channel_multiplier=-1,
                    )
                for hh in range(2):
                    nc.tensor.matmul(
                        po2[hh][:, P * m : 512],
                        lhsT=v_t[b][i // 4][:, i % 4, hh, :],
                        rhs=pt[:, hh, P * m : 512],
                        start=(i == 0),
                        stop=(i == nkb - 1),
                        skip_group_check=True,
                    )
                if feed and (i % 2 == 1):
                    pop_units(1)
            for hh in range(2):
                po = po2[hh]
                if _OLD_DRAIN:
                    lrow = misc.tile([1, 512], FP, tag="lrow")
                    nc.vector.tensor_copy(lrow, po[HD : HD + 1, :])
                    rec = misc.tile([1, 512], FP, tag="rec")
                    nc.vector.reciprocal_approx_fast(rec, lrow)
                    bcast = misc.tile([HD, 512], FP, tag="bcast")
                    nc.gpsimd.partition_broadcast(bcast, rec)
                    at = outp.tile([HD, 512], BF, tag="at")
                    nc.vector.tensor_mul(at, po[0:HD, :], bcast)
                else:
                    # copy out of PSUM promptly to release the po bank early;
                    # reciprocal_approx_fast needs an SBUF src at partition 0
                    lrow = misc.tile([1, 512], FP, tag="lrow")
                    nc.vector.tensor_copy(lrow, po[HD : HD + 1, :])
                    posb = outp.tile([HD, 512], FP, tag="posb")
                    nc.vector.tensor_copy(posb, po[0:HD, :])
                    rec = misc.tile([1, 512], FP, tag="rec")
                    nc.vector.reciprocal_approx_fast(rec, lrow)
                    bcast = misc.tile([HD, 512], FP, tag="bcast")
                    nc.gpsimd.partition_broadcast(bcast, rec)
                    at = outp.tile([HD, 512], BF, tag="at")
                    nc.vector.tensor_mul(at, posb, bcast)
                nc.sync.dma_start(a2a_in[ph][2 * b + j // 2, ds(HD * hh, HD), :], at)

        # phase A (within-half cols 0-511), QKV units fed into the stream
        flush_units(len(units)) if _NO_FEED else None
        for b in range(B):
            for j in (0, 2):
                emit_attention(b, j, 0, feed=not _NO_FEED)
        nc.gpsimd.collective_compute(
            "AllToAll",
            mybir.AluOpType.bypass,
            replica_groups=[list(range(8))],
            ins=[a2a_in[0][:].opt()],
            outs=[a2a_out[0][:].opt()],
        )
        # phase B (within-half cols 512-1023)
        for b in range(B):
            for j in (1, 3):
                emit_attention(b, j, 1, feed=not _NO_FEED)
        flush_units(len(units))
        nc.gpsimd.collective_compute(
            "AllToAll",
            mybir.AluOpType.bypass,
            replica_groups=[list(range(8))],
            ins=[a2a_in[1][:].opt()],
            outs=[a2a_out[1][:].opt()],
        )

        # ---- out projection for my 1024-token group ----------------------
        wout = big.tile([P, CB, C], BF)
        nc.sync.dma_start(wout, wout_p.rearrange("(cb p) c -> p cb c", p=P))
        bout = const.tile([1, C], BF)
        nc.sync.dma_start(bout, bout_p[:])
        boutb = const.tile([P, C], BF)
        nc.gpsimd.partition_broadcast(boutb, bout)
        at_all = big.tile([P, CB, TOK_G], BF)
        for ph in range(2):
            nc.sync.dma_start(
                at_all[:, :, ts(ph, 512)], a2a_out[ph].rearrange("i p t -> p i t")
            )
        for tb in range(TOK_G // P):
            for co in range(2):
                py = psk_pool.tile([P, 512], FP, tag="psk")
                for kb in range(CB):
                    nc.tensor.matmul(
                        py,
                        lhsT=at_all[:, kb, ts(tb, P)],
                        rhs=wout[:, kb, ts(co, 512)],
                        start=(kb == 0),
                        stop=(kb == CB - 1),
                        skip_group_check=True,
                    )
                ot = outp.tile([P, 512], FP, tag="ot")
                nc.vector.tensor_add(ot, py, boutb[:, ts(co, 512)])
                nc.sync.dma_start(out_p[ts(tb, P), ts(co, 512)], ot)

    nc.finalize()
    return nc


def _get_nc():
    if "nc" not in _CACHE:
        _CACHE["nc"] = _build_nc()
    return _CACHE["nc"]


def _shard_inputs(x, w_qkv, b_qkv, w_out, b_out):
    x = np.asarray(x, dtype=np.float32)
    w_qkv = np.asarray(w_qkv, dtype=np.float32)
    b_qkv = np.asarray(b_qkv, dtype=np.float32)
    w_out = np.asarray(w_out, dtype=np.float32)
    b_out = np.asarray(b_out, dtype=np.float32)

    xT = np.ascontiguousarray(x.transpose(0, 2, 1)).astype(BF16)  # (B, C, N)
    wout_b = np.ascontiguousarray(w_out).astype(BF16)
    bout_r = np.ascontiguousarray(b_out[None, :]).astype(BF16)

    in_maps = []
    for c in range(8):
        h0 = NHC * c  # first head owned by this core
        c0 = HD * h0
        wq = w_qkv[:, 0 * C + c0 : 0 * C + c0 + VC]
        wk = w_qkv[:, 1 * C + c0 : 1 * C + c0 + VC]
        wvv = w_qkv[:, 2 * C + c0 : 2 * C + c0 + VC]
        bq = b_qkv[0 * C + c0 : 0 * C + c0 + VC]
        bk = b_qkv[1 * C + c0 : 1 * C + c0 + VC]
        bvv = b_qkv[2 * C + c0 : 2 * C + c0 + VC]
        in_maps.append(
            dict(
                xT=xT,
                wqk=np.ascontiguousarray(np.concatenate([wq, wk], axis=1)).astype(BF16),
                wv=np.ascontiguousarray(wvv).astype(BF16),
                wout=wout_b,
                bqk=np.ascontiguousarray(
                    np.stack([bq, bk], axis=1)
                ).astype(np.float32),
                bv=np.ascontiguousarray(bvv[None, :]).astype(BF16),
                bout=bout_r,
            )
        )
    return in_maps


def kernel(x, attention_mask, w_qkv, b_qkv, w_out, b_out):
    from concourse.bass_utils import run_bass_kernel_spmd

    nc = _get_nc()
    in_maps = _shard_inputs(x, w_qkv, b_qkv, w_out, b_out)
    res = run_bass_kernel_spmd(nc, in_maps, core_ids=list(range(8)))
    _CACHE["last_results"] = res
    out = np.empty((B, N, C), np.float32)
    for c in range(8):
        b = c // 2
        t0 = (c % 2) * TOK_G
        out[b, t0 : t0 + TOK_G] = np.asarray(res.results[c]["out"])
    return out


# revision 22
# speedup vs baseline: 1.0425x; 1.0425x over previous
"""Trainium2 8-core kernel for causal multi-head attention block.

Module: qkv = x @ w_qkv + b_qkv ; causal MHA (16 heads, hd=64) ; out = attn @ w_out + b_out
Shapes: x (4, 2048, 1024), out (4, 2048, 1024), f32.

Sharding (8 cores): tensor-parallel over heads — core c owns heads {2c, 2c+1}
for ALL batches (QKV columns sharded head-wise). After attention, an 8-way
AllToAll converts head-sharding to token-sharding: token group of core c is
(batch c//2, token half c%2). Each core then runs the out projection for its
1024 tokens and outputs its (1024, 1024) f32 slice; the host reassembles.

Device algorithm per core (matmuls bf16, f32 PSUM accumulation):
  1. Q^T, K^T (head-cols on partitions) and V (tokens on partitions, with a
     constant ones-column appended at col 65) from streamed x^T tiles
  2. scores S^T[k,q] = K^T.T @ Q^T per head (2 heads packed in the PE array
     via row tiling, K=64 each); exp on ScalarE with the 1/8 softmax scale
     folded in; causal mask via gpsimd memset/affine_select on diagonal tiles
  3. PV: out^T[hd,q] + l[q] (row 64, from the ones-column of V) accumulated
     over k-blocks in PSUM; normalize by 1/l; bf16 to the AllToAll buffer
  4. two AllToAlls (one per within-half column chunk) so comm overlaps the
     second half of attention and the out projection
  5. out projection for my token group + bias (bias via K=1 matmul), f32 out
"""

import os
import numpy as np
import ml_dtypes

B, N, C, H, HD = 4, 2048, 1024, 16, 64
SCALE = HD ** -0.5
P = 128
CB = C // P               # 8 contraction blocks
NKB = N // P              # 16 key blocks per batch
NQC = N // 512            # 4 query chunks per batch
TOK_G = 1024              # tokens per core after A2A (output slice rows)
NHC = 2                   # heads per core
VC = NHC * HD             # 128 v columns per core

BF16 = ml_dtypes.bfloat16

_CACHE = {}
_NO_FEED = bool(int(os.environ.get("KERNEL_NO_FEED", "0")))
_OLD_DRAIN = bool(int(os.environ.get("KERNEL_OLD_DRAIN", "0")))


def _build_nc():
    import concourse.bass as bass
    import concourse.tile as tile
    from concourse import bacc, mybir
    from concourse.bass import ts, ds
    from contextlib import ExitStack

    FP = mybir.dt.float32
    BF = mybir.dt.bfloat16
    EXP = mybir.ActivationFunctionType.Exp
    IDENT = mybir.ActivationFunctionType.Identity

    nc = bacc.Bacc(num_devices=8)

    # per-core inputs
    xT_p = nc.declare_dram_parameter("xT", [B, C, N], BF, isOutput=False)
    wqk_p = nc.declare_dram_parameter("wqk", [C, 2 * VC], BF, isOutput=False)
    wv_p = nc.declare_dram_parameter("wv", [C, VC], BF, isOutput=False)
    wout_p = nc.declare_dram_parameter("wout", [C, C], BF, isOutput=False)
    bqk_p = nc.declare_dram_parameter("bqk", [P, 2], FP, isOutput=False)
    bv_p = nc.declare_dram_parameter("bv", [1, VC], BF, isOutput=False)
    bout_p = nc.declare_dram_parameter("bout", [1, C], BF, isOutput=False)
    out_p = nc.declare_dram_parameter("out", [TOK_G, C], FP, isOutput=True)

    # A2A buffers: shard s carries my 128 feature rows for core s's tokens.
    # Split into two column phases for comm/compute overlap.
    a2a_in = [
        nc.dram_tensor(f"a2a_in{ph}", [8, P, 512], BF) for ph in range(2)
    ]
    a2a_out = [
        nc.dram_tensor(f"a2a_out{ph}", [8, P, 512], BF) for ph in range(2)
    ]

    with tile.TileContext(nc) as tc, ExitStack() as ctx:
        const = ctx.enter_context(tc.tile_pool(name="const", bufs=1))
        big = ctx.enter_context(tc.tile_pool(name="big", bufs=1))
        xt_pool = ctx.enter_context(tc.tile_pool(name="xtp", bufs=3))
        pt_pool = ctx.enter_context(tc.tile_pool(name="ptp", bufs=6))
        # PSUM budget (8 banks): ps 2x[128,2,512]=4 (QKV/outproj share), po 4x[65,512]=4
        ps_pool = ctx.enter_context(tc.tile_pool(name="psp", bufs=2, space="PSUM"))
        po_pool = ctx.enter_context(tc.tile_pool(name="pop", bufs=4, space="PSUM"))
        misc = ctx.enter_context(tc.tile_pool(name="misc", bufs=6))
        outp = ctx.enter_context(tc.tile_pool(name="outp", bufs=4))

        wqk = const.tile([P, CB, 2 * VC], BF)
        nc.sync.dma_start(wqk, wqk_p.rearrange("(cb p) c -> p cb c", p=P))
        wv = const.tile([P, CB, VC], BF)
        nc.sync.dma_start(wv, wv_p.rearrange("(cb p) c -> p cb c", p=P))
        bqk = const.tile([P, 2], FP)
        nc.sync.dma_start(bqk, bqk_p[:])
        bv = const.tile([1, VC], BF)
        nc.sync.dma_start(bv, bv_p[:])
        bvb = const.tile([P, VC], BF)
        nc.gpsimd.partition_broadcast(bvb, bv)

        # per-(batch, 512-token-chunk) Q^T/K^T and V tiles for fine deps
        qk_t = [
            [big.tile([P, 2, 512], BF, name=f"qkT{b}_{t}") for t in range(NQC)]
            for b in range(B)
        ]
        v_t = []
        for b in range(B):
            row = []
            for t in range(NQC):
                vt = big.tile([P, 4, NHC, HD + 1], BF, name=f"v{b}_{t}")
                nc.vector.memset(vt[:, :, :, HD : HD + 1], 1.0)
                row.append(vt)
            v_t.append(row)

        # ---- QKV work units (one unit = one 8-MM chain + epilogue) -------
        def qkv_unit(b, tch, kind, idx):
            """kind 'qk': idx in (0,1); kind 'v': idx in 0..3"""
            xt = xt_cache.get((b, tch))
            if xt is None:
                xt = xt_pool.tile([P, CB, 512], BF, tag="xt", name=f"xt{b}_{tch}")
                nc.sync.dma_start(
                    xt, xT_p[b, :, ts(tch, 512)].rearrange("(cb p) t -> p cb t", p=P)
                )
                xt_cache[(b, tch)] = xt
            if kind == "qk":
                qk = idx
                psq = ps_pool.tile([P, 512], FP, tag="ps", name=f"psq{b}_{tch}_{qk}")
                for kb in range(CB):
                    nc.tensor.matmul(
                        psq,
                        lhsT=wqk[:, kb, ts(qk, P)],
                        rhs=xt[:, kb, :],
                        start=(kb == 0),
                        stop=(kb == CB - 1),
                        skip_group_check=True,
                    )
                nc.vector.tensor_scalar_add(
                    qk_t[b][tch][:, qk, :], psq, bqk[:, qk : qk + 1]
                )
            else:
                tb4 = idx
                psv = ps_pool.tile([P, 512], FP, tag="ps", name=f"psv{b}_{tch}_{tb4}")
                for kb in range(CB):
                    nc.tensor.matmul(
                        psv[:, :VC],
                        lhsT=xt[:, kb, ts(tb4, P)],
                        rhs=wv[:, kb, :],
                        start=(kb == 0),
                        stop=(kb == CB - 1),
                        skip_group_check=True,
                    )
                nc.vector.tensor_tensor(
                    v_t[b][tch][:, tb4, :, 0:HD],
                    psv[:, :VC].rearrange("p (h d) -> p h d", h=NHC),
                    bvb.rearrange("p (h d) -> p h d", h=NHC),
                    mybir.AluOpType.add,
                )

        xt_cache = {}
        # unit list in order; done_upto[(b, tch)] = index after last unit of (b,tch)
        units = []
        unit_done = {}
        for b in range(B):
            for tch in range(NQC):
                for qk in range(2):
                    units.append((b, tch, "qk", qk))
                for tb4 in range(4):
                    units.append((b, tch, "v", tb4))
                unit_done[(b, tch)] = len(units)
        next_unit = [0]

        def flush_units(upto):
            while next_unit[0] < upto:
                qkv_unit(*units[next_unit[0]])
                next_unit[0] += 1

        def pop_units(n):
            flush_units(min(len(units), next_unit[0] + n))

        # ---- attention ----------------------------------------------------
        def emit_attention(b, j, ph, feed):
            # needs Q chunk j, K/V chunks 0..j of batch b
            flush_units(unit_done[(b, j)])
            po2 = [
                po_pool.tile([HD + 1, 512], FP, tag="po", name=f"po{b}_{j}_{k}")
                for k in range(2)
            ]
            nkb = 4 * j + 4
            for i in range(nkb):
                m = max(0, i - 4 * j)
                q0loc = P * m  # q offset within chunk j
                w = 512 - P * m
                pss = ps_pool.tile([P, 2, 512], FP, tag="ps", name=f"pss{b}_{j}_{i}")
                for hh in range(2):
                    rlo = 64 * hh
                    nc.tensor.matmul(
                        pss[:, hh, P * m : 512],
                        lhsT=qk_t[b][i // 4][:, 1, :][rlo : rlo + 64, ts(i % 4, P)],
                        rhs=qk_t[b][j][:, 0, :][rlo : rlo + 64, ds(q0loc, w)],
                        start=True,
                        stop=True,
                        skip_group_check=True,
                    )
                pt = pt_pool.tile([P, 2, 512], BF, tag="pt", name=f"pt{b}_{j}_{i}")
                nc.scalar.activation(
                    pt[:, :, P * m : 512], pss[:, :, P * m : 512], EXP, scale=SCALE
                )
                if i >= 4 * j:
                    nc.gpsimd.affine_select(
                        out=pt[:, :, P * m : P * m + P],
                        in_=pt[:, :, P * m : P * m + P],
                        compare_op=mybir.AluOpType.is_ge,
                        fill=0.0,
                        base=0,
                        pattern=[[0, 2], [1, P]],
                        channel_multiplier=-1,
                    )
                for hh in range(2):
                    nc.tensor.matmul(
                        po2[hh][:, P * m : 512],
                        lhsT=v_t[b][i // 4][:, i % 4, hh, :],
                        rhs=pt[:, hh, P * m : 512],
                        start=(i == 0),
                        stop=(i == nkb - 1),
                        skip_group_check=True,
                    )
                if feed and (i % 2 == 1):
                    pop_units(1)
            for hh in range(2):
                po = po2[hh]
                if _OLD_DRAIN:
                    lrow = misc.tile([1, 512], FP, tag="lrow")
                    nc.vector.tensor_copy(lrow, po[HD : HD + 1, :])
                    rec = misc.tile([1, 512], FP, tag="rec")
                    nc.vector.reciprocal_approx_fast(rec, lrow)
                    bcast = misc.tile([HD, 512], FP, tag="bcast")
                    nc.gpsimd.partition_broadcast(bcast, rec)
                    at = outp.tile([HD, 512], BF, tag="at")
                    nc.vector.tensor_mul(at, po[0:HD, :], bcast)
                else:
                    # copy out of PSUM promptly to release the po bank early;
                    # reciprocal_approx_fast needs an SBUF src at partition 0
                    lrow = misc.tile([1, 512], FP, tag="lrow")
                    nc.vector.tensor_copy(lrow, po[HD : HD + 1, :])
                    posb = outp.tile([HD, 512], FP, tag="posb")
                    nc.vector.tensor_copy(posb, po[0:HD, :])
                    rec = misc.tile([1, 512], FP, tag="rec")
                    nc.vector.reciprocal_approx_fast(rec, lrow)
                    bcast = misc.tile([HD, 512], FP, tag="bcast")
                    nc.gpsimd.partition_broadcast(bcast, rec)
                    at = outp.tile([HD, 512], BF, tag="at")
                    nc.vector.tensor_mul(at, posb, bcast)
                nc.sync.dma_start(a2a_in[ph][2 * b + j // 2, ds(HD * hh, HD), :], at)

        # phase A (within-half cols 0-511), QKV units fed into the stream
        flush_units(len(units)) if _NO_FEED else None
        for b in range(B):
            for j in (0, 2):
                emit_attention(b, j, 0, feed=not _NO_FEED)
        # phase B (within-half cols 512-1023); A2A(A) is triggered after the
        # first phase-B block so its input DMAs have drained by then
        first_b_done = False
        for b in range(B):
            for j in (1, 3):
                emit_attention(b, j, 1, feed=not _NO_FEED)
                if not first_b_done:
                    first_b_done = True
                    nc.gpsimd.collective_compute(
                        "AllToAll",
                        mybir.AluOpType.bypass,
                        replica_groups=[list(range(8))],
                        ins=[a2a_in[0][:].opt()],
                        outs=[a2a_out[0][:].opt()],
                    )
        flush_units(len(units))
        nc.gpsimd.collective_compute(
            "AllToAll",
            mybir.AluOpType.bypass,
            replica_groups=[list(range(8))],
            ins=[a2a_in[1][:].opt()],
            outs=[a2a_out[1][:].opt()],
        )

        # ---- out projection for my 1024-token group ----------------------
        wout = big.tile([P, CB, C], BF)
        nc.sync.dma_start(wout, wout_p.rearrange("(cb p) c -> p cb c", p=P))
        bout = const.tile([1, C], BF)
        nc.sync.dma_start(bout, bout_p[:])
        boutb = const.tile([P, C], BF)
        nc.gpsimd.partition_broadcast(boutb, bout)
        at_all = []
        for ph in range(2):
            ata = big.tile([P, CB, 512], BF, name=f"at_all{ph}")
            nc.sync.dma_start(ata, a2a_out[ph].rearrange("i p t -> p i t"))
            at_all.append(ata)
        for tb in range(TOK_G // P):
            for co in range(2):
                py = ps_pool.tile([P, 512], FP, tag="ps")
                for kb in range(CB):
                    nc.tensor.matmul(
                        py,
                        lhsT=at_all[tb // 4][:, kb, ts(tb % 4, P)],
                        rhs=wout[:, kb, ts(co, 512)],
                        start=(kb == 0),
                        stop=(kb == CB - 1),
                        skip_group_check=True,
                    )
                ot = outp.tile([P, 512], FP, tag="ot")
                nc.vector.tensor_add(ot, py, boutb[:, ts(co, 512)])
                nc.sync.dma_start(out_p[ts(tb, P), ts(co, 512)], ot)

    nc.finalize()
    return nc


def _get_nc():
    if "nc" not in _CACHE:
        _CACHE["nc"] = _build_nc()
    return _CACHE["nc"]


def _shard_inputs(x, w_qkv, b_qkv, w_out, b_out):
    x = np.asarray(x, dtype=np.float32)
    w_qkv = np.asarray(w_qkv, dtype=np.float32)
    b_qkv = np.asarray(b_qkv, dtype=np.float32)
    w_out = np.asarray(w_out, dtype=np.float32)
    b_out = np.asarray(b_out, dtype=np.float32)

    xT = np.ascontiguousarray(x.transpose(0, 2, 1)).astype(BF16)  # (B, C, N)
    wout_b = np.ascontiguousarray(w_out).astype(BF16)
    bout_r = np.ascontiguousarray(b_out[None, :]).astype(BF16)

    in_maps = []
    for c in range(8):
        h0 = NHC * c  # first head owned by this core
        c0 = HD * h0
        wq = w_qkv[:, 0 * C + c0 : 0 * C + c0 + VC]
        wk = w_qkv[:, 1 * C + c0 : 1 * C + c0 + VC]
        wvv = w_qkv[:, 2 * C + c0 : 2 * C + c0 + VC]
        bq = b_qkv[0 * C + c0 : 0 * C + c0 + VC]
        bk = b_qkv[1 * C + c0 : 1 * C + c0 + VC]
        bvv = b_qkv[2 * C + c0 : 2 * C + c0 + VC]
        in_maps.append(
            dict(
                xT=xT,
                wqk=np.ascontiguousarray(np.concatenate([wq, wk], axis=1)).astype(BF16),
                wv=np.ascontiguousarray(wvv).astype(BF16),
                wout=wout_b,
                bqk=np.ascontiguousarray(
                    np.stack([bq, bk], axis=1)
                ).astype(np.float32),
                bv=np.ascontiguousarray(bvv[None, :]).astype(BF16),
                bout=bout_r,
            )
        )
    return in_maps


def kernel(x, attention_mask, w_qkv, b_qkv, w_out, b_out):
    from concourse.bass_utils import run_bass_kernel_spmd

    nc = _get_nc()
    in_maps = _shard_inputs(x, w_qkv, b_qkv, w_out, b_out)
    res = run_bass_kernel_spmd(nc, in_maps, core_ids=list(range(8)))
    _CACHE["last_results"] = res
    out = np.empty((B, N, C), np.float32)
    for c in range(8):
        b = c // 2
        t0 = (c % 2) * TOK_G
        out[b, t0 : t0 + TOK_G] = np.asarray(res.results[c]["out"])
    return out
